# revision 18
# baseline (speedup 1.0000x reference)
"""Deformable 3x3 conv (AdaptiveConv, N=16 C=256 H=W=32) on 8 trn2 cores.

Data-parallel over batch: 2 samples per core. Per sample, on-chip:
  1. DVE pipeline: offsets -> bilinear entry indices + 4 corner weights
     (with OOB remap so clipped entries get the right weights).
  2. idx/weights bounce via DRAM into gather-wrapped / row layouts.
  3. xdup: row-pair duplicated image in DRAM; entry hw = 4 corner pixels
     (2KB bf16) -> one dma_gather descriptor per sampling position.
  4. Transpose-mode dma_gather -> (channel, position) layout tiles.
  5. DVE lerp (broadcast weight rows) -> im2col val tiles.
  6. TensorE: 18 (c-chunk, k) PSUM-accumulated matmuls -> out (o, hw).
"""
from contextlib import ExitStack

import numpy as np

try:
    import ml_dtypes
    _BF16 = ml_dtypes.bfloat16
except ImportError:  # pragma: no cover
    _BF16 = None

N, C_IN, C_OUT, H, W = 16, 256, 256, 32, 32
K = 9
HW = H * W
NPIX = 1152
NCORES = 8
SPC = N // NCORES

_cache = {}


def _build():
    import concourse.bass as bass
    import concourse.mybir as mybir
    import concourse.tile as tile
    from concourse import bacc

    bf = mybir.dt.bfloat16
    f32 = mybir.dt.float32
    i16 = mybir.dt.int16
    AOT = mybir.AluOpType

    nc = bacc.Bacc("TRN2", target_bir_lowering=False, debug=False,
                   num_devices=NCORES)

    xb = nc.declare_dram_parameter("xb", [SPC, NPIX, C_IN], bf, isOutput=False)
    off_w = nc.declare_dram_parameter("off_w", [SPC, 128, 8, 2 * K], f32,
                                      isOutput=False)
    base_w = nc.declare_dram_parameter("base_w", [128, 8, 2 * K], f32,
                                       isOutput=False)
    wt = nc.declare_dram_parameter("wt", [2 * K, 128, C_OUT], bf,
                                   isOutput=False)
    out_d = nc.declare_dram_parameter("out", [SPC, C_OUT, HW], f32,
                                      isOutput=True)

    xdup = nc.dram_tensor("xdup", [SPC, HW, 4 * C_IN], bf)
    istage = nc.dram_tensor("istage", [SPC, HW, K], i16)

    with tile.TileContext(nc) as tc, ExitStack() as ctx:
        cpool = ctx.enter_context(tc.tile_pool(name="const", bufs=1))
        ppool = ctx.enter_context(tc.tile_pool(name="pipe", bufs=2))
        tpool = ctx.enter_context(tc.tile_pool(name="tmp", bufs=2))
        gpool = ctx.enter_context(tc.tile_pool(name="gath", bufs=2))
        vpool = ctx.enter_context(tc.tile_pool(name="val", bufs=2))
        opool = ctx.enter_context(tc.tile_pool(name="outs", bufs=2))
        pspool = ctx.enter_context(
            tc.tile_pool(name="psum", bufs=1, space="PSUM"))
        trpool = ctx.enter_context(
            tc.tile_pool(name="trp", bufs=2, space="PSUM"))

        from concourse.masks import make_identity
        ident = cpool.tile([128, 128], bf)
        make_identity(nc, ident[:])
        baset = cpool.tile([128, 8, 2 * K], f32)
        nc.sync.dma_start(baset[:], base_w[:, :, :])
        wtt = cpool.tile([128, 2 * K, C_OUT], bf)
        nc.sync.dma_start(wtt[:], wt[:, :, :].rearrange("m i o -> i m o"))

        prep = {}
        for s in range(SPC):
            # ---- xdup: entry hw -> [(y,x) (y,x+1) (y+1,x) (y+1,x+1)] ----
            xs = xb[s]
            for half in range(2):
                src = bass.AP(xs.tensor, xs.offset + half * W * C_IN,
                              [[C_IN, HW], [1, 2 * C_IN]])
                eng = nc.scalar if half == 0 else nc.sync
                eng.dma_start(
                    xdup[s][:, half * 2 * C_IN:(half + 1) * 2 * C_IN], src)

            # ---- pipeline: (128, 8, 18) wrapped layout ----
            SH = [128, 8, 2 * K]
            offt = ppool.tile(SH, f32, tag="off")
            nc.sync.dma_start(offt[:], off_w[s])
            py = tpool.tile(SH, f32, tag="py")
            nc.vector.tensor_tensor(py[:], offt[:], baset[:], AOT.add)
            fli = tpool.tile(SH, mybir.dt.int16, tag="fli")
            nc.vector.tensor_copy(fli[:], py[:])
            cf = tpool.tile(SH, f32, tag="cf")
            nc.vector.tensor_copy(cf[:], fli[:])
            gg = tpool.tile(SH, f32, tag="gg")
            nc.vector.tensor_tensor(gg[:], cf[:], py[:], AOT.is_gt)
            fl = tpool.tile(SH, f32, tag="fl")
            nc.vector.tensor_tensor(fl[:], cf[:], gg[:], AOT.subtract)
            frac = tpool.tile(SH, f32, tag="frac")
            nc.vector.tensor_tensor(frac[:], py[:], fl[:], AOT.subtract)
            a = tpool.tile(SH, f32, tag="a")           # [fl >= 0]
            nc.vector.tensor_scalar(a[:], fl[:], 0.0, None, AOT.is_ge)
            vb = tpool.tile(SH, f32, tag="vb")
            nc.vector.tensor_scalar(vb[:], fl[:], 31.0, None, AOT.is_le)
            v0 = tpool.tile(SH, f32, tag="v0")         # fl in [0,31]
            nc.vector.tensor_tensor(v0[:], a[:], vb[:], AOT.mult)
            va = tpool.tile(SH, f32, tag="va")
            nc.vector.tensor_scalar(va[:], fl[:], -1.0, None, AOT.is_ge)
            nc.vector.tensor_scalar(vb[:], fl[:], 30.0, None, AOT.is_le)
            v1 = tpool.tile(SH, f32, tag="v1")         # fl+1 in [0,31]
            nc.vector.tensor_tensor(v1[:], va[:], vb[:], AOT.mult)
            flc = tpool.tile(SH, f32, tag="flc")
            nc.vector.tensor_scalar(flc[:], fl[:], 0.0, 31.0, AOT.max,
                                    AOT.min)
            om = tpool.tile(SH, f32, tag="om")         # 1 - frac
            nc.vector.tensor_scalar(om[:], frac[:], -1.0, 1.0, AOT.mult,
                                    AOT.add)
            w0 = tpool.tile(SH, f32, tag="w0")         # lo-corner weight
            nc.vector.tensor_tensor(w0[:], om[:], v0[:], AOT.mult)
            w1 = tpool.tile(SH, f32, tag="w1")         # hi-corner weight
            nc.vector.tensor_tensor(w1[:], frac[:], v1[:], AOT.mult)
            # OOB remap onto clipped entry: wP0 = a*w0 + (1-a)*w1, wP1 = a*w1
            na = tpool.tile(SH, f32, tag="na")
            nc.vector.tensor_scalar(na[:], a[:], -1.0, 1.0, AOT.mult, AOT.add)
            t0 = tpool.tile(SH, f32, tag="t0")
            nc.vector.tensor_tensor(t0[:], a[:], w0[:], AOT.mult)
            t1 = tpool.tile(SH, f32, tag="t1")
            nc.vector.tensor_tensor(t1[:], na[:], w1[:], AOT.mult)
            wp0 = tpool.tile(SH, f32, tag="wp0")
            nc.vector.tensor_tensor(wp0[:], t0[:], t1[:], AOT.add)
            wp1 = tpool.tile(SH, f32, tag="wp1")
            nc.vector.tensor_tensor(wp1[:], a[:], w1[:], AOT.mult)

            wplane = ppool.tile([128, 8, K, 4], f32, tag="wplane")
            wy0, wx0 = wp0[:, :, 0::2], wp0[:, :, 1::2]
            wy1, wx1 = wp1[:, :, 0::2], wp1[:, :, 1::2]
            nc.vector.tensor_tensor(wplane[:, :, :, 0], wy0, wx0, AOT.mult)
            nc.vector.tensor_tensor(wplane[:, :, :, 1], wy0, wx1, AOT.mult)
            nc.vector.tensor_tensor(wplane[:, :, :, 2], wy1, wx0, AOT.mult)
            nc.vector.tensor_tensor(wplane[:, :, :, 3], wy1, wx1, AOT.mult)

            t2 = tpool.tile([128, 8, K], f32, tag="t2")
            nc.vector.tensor_scalar(t2[:], flc[:, :, 0::2], float(W), None,
                                    AOT.mult)
            idxp = ppool.tile([128, 8, K], i16, tag="idxp")
            nc.vector.tensor_tensor(idxp[:], t2[:], flc[:, :, 1::2], AOT.add)

            # ---- bounce idx/weights through DRAM (hw-major staging) ----
            ist = istage[s]  # (HW, K): addr = hw*K + k
            nc.scalar.dma_start(
                bass.AP(ist.tensor, ist.offset,
                        [[K, 128], [128 * K, 8], [1, K]]),
                idxp[:])
            # wrapped idx: load (16, 64, K) then ACT repack+replicate per group
            idxwt = ppool.tile([16, HW // 16, K], i16, tag="idxwt")
            nc.scalar.dma_start(
                idxwt[:],
                bass.AP(ist.tensor, ist.offset,
                        [[K, 16], [16 * K, HW // 16], [1, K]]))
            idxw16 = ppool.tile([16, K, HW // 16], i16, tag="idxw16")
            nc.scalar.copy(
                idxw16[:],
                bass.AP(idxwt.tensor, idxwt.offset,
                        [[idxwt.ap[0][0], 16], [1, K], [K, HW // 16]]))
            idxw = ppool.tile([128, K, HW // 16], i16, tag="idxw")
            for grp in range(8):
                nc.scalar.dma_start(idxw[grp * 16:(grp + 1) * 16], idxw16[:])

            prep[s] = (idxw, wplane)

        for s in range(SPC):
            idxw, wplane = prep[s]
            ps = {}
            for oc in range(2):
                for hwin in range(2):
                    pst = pspool.tile([128, 512], f32, tag=f"ps{oc}{hwin}")
                    ps[(oc, hwin)] = pst

            # ---- gather 2 taps per call; lerp + PE transpose per tap ----
            for k0 in range(0, K, 2):
                nk = min(2, K - k0)
                g2 = gpool.tile([128, 16, 4 * C_IN], bf, tag="g2")
                nc.gpsimd.dma_gather(
                    out_ap=g2[:, :8 * nk, :],
                    in_ap=xdup[s],
                    idxs_ap=idxw[:, k0:k0 + nk, :],
                    num_idxs=nk * HW,
                    num_idxs_reg=nk * HW,
                    elem_size=4 * C_IN,
                    transpose=False,
                    single_packet=False,
                )
                for k in range(k0, k0 + nk):
                    g = g2[:, 8 * (k - k0):8 * (k - k0) + 8, :]
                    ptr = {}
                    for cc in range(2):
                        pt = trpool.tile([128, 8, 128], bf, tag=f"tr{cc}")
                        ptr[cc] = pt
                    # batched lerp: one bcast-mult + 3 adds for the whole k
                    wsl = wplane[:, :, k, :]
                    wb = bass.AP(wsl.tensor, wsl.offset,
                                 list(wsl.ap) + [[0, C_IN]])
                    mall = vpool.tile([128, 8, 4, C_IN], bf, tag="mall")
                    nc.vector.tensor_tensor(
                        mall[:],
                        g.rearrange("p b (c4 c) -> p b c4 c", c=C_IN),
                        wb, AOT.mult)
                    a1 = vpool.tile([128, 8, C_IN], bf, tag="a1")
                    nc.vector.tensor_tensor(a1[:], mall[:, :, 0, :],
                                            mall[:, :, 1, :], AOT.add)
                    a2 = vpool.tile([128, 8, C_IN], bf, tag="a2")
                    nc.vector.tensor_tensor(a2[:], mall[:, :, 2, :],
                                            mall[:, :, 3, :], AOT.add)
                    vall = vpool.tile([128, 8, C_IN], bf, tag="vall")
                    nc.vector.tensor_tensor(vall[:], a1[:], a2[:], AOT.add)
                    for b in range(8):
                        for cc in range(2):
                            nc.tensor.transpose(
                                ptr[cc][:, b, :],
                                vall[:, b, cc * 128:(cc + 1) * 128], ident[:])
                    for cc in range(2):
                        valt = vpool.tile([128, HW], bf, tag="valt")
                        nc.scalar.copy(
                            valt[:], ptr[cc][:].rearrange("p a b -> p (a b)"))
                        for oc in range(2):
                            for hwin in range(2):
                                nc.tensor.matmul(
                                    ps[(oc, hwin)][:],
                                    lhsT=wtt[:, cc * K + k,
                                             oc * 128:(oc + 1) * 128],
                                    rhs=valt[:, hwin * 512:(hwin + 1) * 512],
                                    start=(k == 0 and cc == 0),
                                    stop=(k == K - 1 and cc == 1),
                                )

            for oc in range(2):
                ot = opool.tile([128, HW], f32, tag="ot")
                for hwin in range(2):
                    nc.scalar.copy(ot[:, hwin * 512:(hwin + 1) * 512],
                                   ps[(oc, hwin)][:])
                nc.sync.dma_start(out_d[s][oc * 128:(oc + 1) * 128, :], ot[:])

    nc.compile()
    return nc


def get_nc():
    if "nc" not in _cache:
        _cache["nc"] = _build()
    return _cache["nc"]


def prep_core_inputs(x, offset, weight, core):
    """Host-side shard + layout for one core. x (N,C,H,W) f32,
    offset (N,HW,18) f32, weight (O,C,3,3) f32."""
    s0 = core * SPC
    xbs = np.zeros((SPC, NPIX, C_IN), dtype=_BF16)
    offw = np.empty((SPC, 128, 8, 2 * K), dtype=np.float32)
    for i, s in enumerate(range(s0, s0 + SPC)):
        xt = x[s].reshape(C_IN, HW).T.astype(_BF16)  # (1024, 256)
        xbs[i, :HW] = xt
        offw[i] = offset[s].reshape(8, 128, 2 * K).transpose(1, 0, 2)
    return {"xb": xbs, "off_w": offw}


def make_base_w():
    hwv = (np.arange(8)[None, :] * 128 + np.arange(128)[:, None])  # (128,8)
    ky = np.arange(K) // 3 - 1
    kx = np.arange(K) % 3 - 1
    base = np.empty((128, 8, 2 * K), dtype=np.float32)
    base[:, :, 0::2] = (hwv // W)[:, :, None] + ky[None, None, :]
    base[:, :, 1::2] = (hwv % W)[:, :, None] + kx[None, None, :]
    return base


def make_wt(weight):
    wk = weight.reshape(C_OUT, C_IN, K)  # (O, C, K)
    wt = np.empty((2 * K, 128, C_OUT), dtype=_BF16)
    for cc in range(2):
        for k in range(K):
            wt[cc * K + k] = wk[:, cc * 128:(cc + 1) * 128, k].T
    return wt


def _ensure_device():
    import subprocess
    probe = (
        "import jax, numpy as np; "
        "x = jax.device_put(np.ones((4,4), np.float32), jax.devices()[0]); "
        "print('probe:', float((x+1).sum()))"
    )
    reset = (
        "import ctypes, jax, time; jax.devices(); "
        "lib = ctypes.CDLL('/opt/axon/libaxon_pjrt.so'); "
        "lib.axon_reset.restype = ctypes.c_int64; "
        "print('rc', lib.axon_reset()); time.sleep(2)"
    )
    import sys as _sys
    r = subprocess.run([_sys.executable, "-c", probe], capture_output=True,
                       text=True, timeout=300)
    if "probe: 32.0" in r.stdout:
        return
    for _ in range(3):
        subprocess.run([_sys.executable, "-c", reset], timeout=300)
        r = subprocess.run([_sys.executable, "-c", probe],
                           capture_output=True, text=True, timeout=300)
        if "probe: 32.0" in r.stdout:
            return


def kernel(x, offset, weight):
    from concourse.bass_utils import run_bass_kernel_spmd

    _ensure_device()

    x = np.asarray(x, dtype=np.float32)
    offset = np.asarray(offset, dtype=np.float32)
    weight = np.asarray(weight, dtype=np.float32)
    nc = get_nc()
    base = make_base_w()
    wt = make_wt(weight)
    in_maps = []
    for c in range(NCORES):
        m = prep_core_inputs(x, offset, weight, c)
        m["base_w"] = base
        m["wt"] = wt
        in_maps.append(m)
    res = run_bass_kernel_spmd(nc, in_maps, core_ids=list(range(NCORES)))
    out = np.empty((N, C_OUT, H, W), dtype=np.float32)
    for c in range(NCORES):
        o = np.asarray(res.results[c]["out"], dtype=np.float32)
        out[c * SPC:(c + 1) * SPC] = o.reshape(SPC, C_OUT, H, W)
    return out


# revision 19
# speedup vs baseline: 1.1161x; 1.1161x over previous
"""Deformable 3x3 conv (AdaptiveConv, N=16 C=256 H=W=32) on 8 trn2 cores.

Data-parallel over batch: 2 samples per core. Per sample, on-chip:
  1. DVE pipeline: offsets -> bilinear entry indices + 4 corner weights
     (with OOB remap so clipped entries get the right weights).
  2. idx/weights bounce via DRAM into gather-wrapped / row layouts.
  3. xdup: row-pair duplicated image in DRAM; entry hw = 4 corner pixels
     (2KB bf16) -> one dma_gather descriptor per sampling position.
  4. Transpose-mode dma_gather -> (channel, position) layout tiles.
  5. DVE lerp (broadcast weight rows) -> im2col val tiles.
  6. TensorE: 18 (c-chunk, k) PSUM-accumulated matmuls -> out (o, hw).
"""
from contextlib import ExitStack

import numpy as np

try:
    import ml_dtypes
    _BF16 = ml_dtypes.bfloat16
except ImportError:  # pragma: no cover
    _BF16 = None

N, C_IN, C_OUT, H, W = 16, 256, 256, 32, 32
K = 9
HW = H * W
NPIX = 1152
NCORES = 8
SPC = N // NCORES

_cache = {}


def _build():
    import concourse.bass as bass
    import concourse.mybir as mybir
    import concourse.tile as tile
    from concourse import bacc

    bf = mybir.dt.bfloat16
    f32 = mybir.dt.float32
    i16 = mybir.dt.int16
    AOT = mybir.AluOpType

    nc = bacc.Bacc("TRN2", target_bir_lowering=False, debug=False,
                   num_devices=NCORES)

    xb = nc.declare_dram_parameter("xb", [SPC, NPIX, C_IN], bf, isOutput=False)
    off_w = nc.declare_dram_parameter("off_w", [SPC, 128, 8, 2 * K], f32,
                                      isOutput=False)
    base_w = nc.declare_dram_parameter("base_w", [128, 8, 2 * K], f32,
                                       isOutput=False)
    wt = nc.declare_dram_parameter("wt", [2 * K, 128, C_OUT], bf,
                                   isOutput=False)
    out_d = nc.declare_dram_parameter("out", [SPC, C_OUT, HW], f32,
                                      isOutput=True)

    xdup = nc.dram_tensor("xdup", [SPC, HW, 4 * C_IN], bf)
    istage = nc.dram_tensor("istage", [SPC, HW, K], i16)

    with tile.TileContext(nc) as tc, ExitStack() as ctx:
        cpool = ctx.enter_context(tc.tile_pool(name="const", bufs=1))
        ppool = ctx.enter_context(tc.tile_pool(name="pipe", bufs=2))
        tpool = ctx.enter_context(tc.tile_pool(name="tmp", bufs=2))
        gpool = ctx.enter_context(tc.tile_pool(name="gath", bufs=4))
        vpool = ctx.enter_context(tc.tile_pool(name="val", bufs=2))
        opool = ctx.enter_context(tc.tile_pool(name="outs", bufs=2))
        pspool = ctx.enter_context(
            tc.tile_pool(name="psum", bufs=1, space="PSUM"))
        trpool = ctx.enter_context(
            tc.tile_pool(name="trp", bufs=2, space="PSUM"))

        from concourse.masks import make_identity
        ident = cpool.tile([128, 128], bf)
        make_identity(nc, ident[:])
        baset = cpool.tile([128, 8, 2 * K], f32)
        nc.sync.dma_start(baset[:], base_w[:, :, :])
        wtt = cpool.tile([128, 2 * K, C_OUT], bf)
        nc.sync.dma_start(wtt[:], wt[:, :, :].rearrange("m i o -> i m o"))

        prep = {}
        for s in range(SPC):
            # ---- xdup: entry hw -> [(y,x) (y,x+1) (y+1,x) (y+1,x+1)] ----
            xs = xb[s]
            for half in range(2):
                src = bass.AP(xs.tensor, xs.offset + half * W * C_IN,
                              [[C_IN, HW], [1, 2 * C_IN]])
                eng = nc.scalar if half == 0 else nc.sync
                eng.dma_start(
                    xdup[s][:, half * 2 * C_IN:(half + 1) * 2 * C_IN], src)

            # ---- pipeline: (128, 8, 18) wrapped layout ----
            SH = [128, 8, 2 * K]
            offt = ppool.tile(SH, f32, tag="off")
            nc.sync.dma_start(offt[:], off_w[s])
            py = tpool.tile(SH, f32, tag="py")
            nc.vector.tensor_tensor(py[:], offt[:], baset[:], AOT.add)
            fli = tpool.tile(SH, mybir.dt.int16, tag="fli")
            nc.vector.tensor_copy(fli[:], py[:])
            cf = tpool.tile(SH, f32, tag="cf")
            nc.vector.tensor_copy(cf[:], fli[:])
            gg = tpool.tile(SH, f32, tag="gg")
            nc.vector.tensor_tensor(gg[:], cf[:], py[:], AOT.is_gt)
            fl = tpool.tile(SH, f32, tag="fl")
            nc.vector.tensor_tensor(fl[:], cf[:], gg[:], AOT.subtract)
            frac = tpool.tile(SH, f32, tag="frac")
            nc.vector.tensor_tensor(frac[:], py[:], fl[:], AOT.subtract)
            a = tpool.tile(SH, f32, tag="a")           # [fl >= 0]
            nc.vector.tensor_scalar(a[:], fl[:], 0.0, None, AOT.is_ge)
            vb = tpool.tile(SH, f32, tag="vb")
            nc.vector.tensor_scalar(vb[:], fl[:], 31.0, None, AOT.is_le)
            v0 = tpool.tile(SH, f32, tag="v0")         # fl in [0,31]
            nc.vector.tensor_tensor(v0[:], a[:], vb[:], AOT.mult)
            va = tpool.tile(SH, f32, tag="va")
            nc.vector.tensor_scalar(va[:], fl[:], -1.0, None, AOT.is_ge)
            nc.vector.tensor_scalar(vb[:], fl[:], 30.0, None, AOT.is_le)
            v1 = tpool.tile(SH, f32, tag="v1")         # fl+1 in [0,31]
            nc.vector.tensor_tensor(v1[:], va[:], vb[:], AOT.mult)
            flc = tpool.tile(SH, f32, tag="flc")
            nc.vector.tensor_scalar(flc[:], fl[:], 0.0, 31.0, AOT.max,
                                    AOT.min)
            om = tpool.tile(SH, f32, tag="om")         # 1 - frac
            nc.vector.tensor_scalar(om[:], frac[:], -1.0, 1.0, AOT.mult,
                                    AOT.add)
            w0 = tpool.tile(SH, f32, tag="w0")         # lo-corner weight
            nc.vector.tensor_tensor(w0[:], om[:], v0[:], AOT.mult)
            w1 = tpool.tile(SH, f32, tag="w1")         # hi-corner weight
            nc.vector.tensor_tensor(w1[:], frac[:], v1[:], AOT.mult)
            # OOB remap onto clipped entry: wP0 = a*w0 + (1-a)*w1, wP1 = a*w1
            na = tpool.tile(SH, f32, tag="na")
            nc.vector.tensor_scalar(na[:], a[:], -1.0, 1.0, AOT.mult, AOT.add)
            t0 = tpool.tile(SH, f32, tag="t0")
            nc.vector.tensor_tensor(t0[:], a[:], w0[:], AOT.mult)
            t1 = tpool.tile(SH, f32, tag="t1")
            nc.vector.tensor_tensor(t1[:], na[:], w1[:], AOT.mult)
            wp0 = tpool.tile(SH, f32, tag="wp0")
            nc.vector.tensor_tensor(wp0[:], t0[:], t1[:], AOT.add)
            wp1 = tpool.tile(SH, f32, tag="wp1")
            nc.vector.tensor_tensor(wp1[:], a[:], w1[:], AOT.mult)

            wplane = ppool.tile([128, 8, K, 4], f32, tag="wplane")
            wy0, wx0 = wp0[:, :, 0::2], wp0[:, :, 1::2]
            wy1, wx1 = wp1[:, :, 0::2], wp1[:, :, 1::2]
            nc.vector.tensor_tensor(wplane[:, :, :, 0], wy0, wx0, AOT.mult)
            nc.vector.tensor_tensor(wplane[:, :, :, 1], wy0, wx1, AOT.mult)
            nc.vector.tensor_tensor(wplane[:, :, :, 2], wy1, wx0, AOT.mult)
            nc.vector.tensor_tensor(wplane[:, :, :, 3], wy1, wx1, AOT.mult)

            t2 = tpool.tile([128, 8, K], f32, tag="t2")
            nc.vector.tensor_scalar(t2[:], flc[:, :, 0::2], float(W), None,
                                    AOT.mult)
            idxp = ppool.tile([128, 8, K], i16, tag="idxp")
            nc.vector.tensor_tensor(idxp[:], t2[:], flc[:, :, 1::2], AOT.add)

            # ---- bounce idx/weights through DRAM (hw-major staging) ----
            ist = istage[s]  # (HW, K): addr = hw*K + k
            nc.scalar.dma_start(
                bass.AP(ist.tensor, ist.offset,
                        [[K, 128], [128 * K, 8], [1, K]]),
                idxp[:])
            # wrapped idx: load (16, 64, K) then ACT repack+replicate per group
            idxwt = ppool.tile([16, HW // 16, K], i16, tag="idxwt")
            nc.scalar.dma_start(
                idxwt[:],
                bass.AP(ist.tensor, ist.offset,
                        [[K, 16], [16 * K, HW // 16], [1, K]]))
            idxw16 = ppool.tile([16, K, HW // 16], i16, tag="idxw16")
            nc.scalar.copy(
                idxw16[:],
                bass.AP(idxwt.tensor, idxwt.offset,
                        [[idxwt.ap[0][0], 16], [1, K], [K, HW // 16]]))
            idxw = ppool.tile([128, K, HW // 16], i16, tag="idxw")
            for grp in range(8):
                nc.scalar.dma_start(idxw[grp * 16:(grp + 1) * 16], idxw16[:])

            prep[s] = (idxw, wplane)

        for s in range(SPC):
            idxw, wplane = prep[s]
            ps = {}
            for oc in range(2):
                for hwin in range(2):
                    pst = pspool.tile([128, 512], f32, tag=f"ps{oc}{hwin}")
                    ps[(oc, hwin)] = pst

            # ---- per-k gather; lerp + PE transpose per tap ----
            for k0 in range(0, K):
                g2 = gpool.tile([128, 8, 4 * C_IN], bf, tag="g2")
                nc.gpsimd.dma_gather(
                    out_ap=g2[:],
                    in_ap=xdup[s],
                    idxs_ap=idxw[:, k0, :],
                    num_idxs=HW,
                    num_idxs_reg=HW,
                    elem_size=4 * C_IN,
                    transpose=False,
                    single_packet=False,
                )
                for k in (k0,):
                    g = g2[:]
                    ptr = {}
                    for cc in range(2):
                        pt = trpool.tile([128, 8, 128], bf, tag=f"tr{cc}")
                        ptr[cc] = pt
                    # batched lerp: one bcast-mult + 3 adds for the whole k
                    wsl = wplane[:, :, k, :]
                    wb = bass.AP(wsl.tensor, wsl.offset,
                                 list(wsl.ap) + [[0, C_IN]])
                    mall = vpool.tile([128, 8, 4, C_IN], bf, tag="mall")
                    nc.vector.tensor_tensor(
                        mall[:],
                        g.rearrange("p b (c4 c) -> p b c4 c", c=C_IN),
                        wb, AOT.mult)
                    a1 = vpool.tile([128, 8, C_IN], bf, tag="a1")
                    nc.vector.tensor_tensor(a1[:], mall[:, :, 0, :],
                                            mall[:, :, 1, :], AOT.add)
                    a2 = vpool.tile([128, 8, C_IN], bf, tag="a2")
                    nc.vector.tensor_tensor(a2[:], mall[:, :, 2, :],
                                            mall[:, :, 3, :], AOT.add)
                    vall = vpool.tile([128, 8, C_IN], bf, tag="vall")
                    nc.vector.tensor_tensor(vall[:], a1[:], a2[:], AOT.add)
                    for b in range(8):
                        for cc in range(2):
                            nc.tensor.transpose(
                                ptr[cc][:, b, :],
                                vall[:, b, cc * 128:(cc + 1) * 128], ident[:])
                    for cc in range(2):
                        valt = vpool.tile([128, HW], bf, tag="valt")
                        nc.scalar.copy(
                            valt[:], ptr[cc][:].rearrange("p a b -> p (a b)"))
                        for oc in range(2):
                            for hwin in range(2):
                                nc.tensor.matmul(
                                    ps[(oc, hwin)][:],
                                    lhsT=wtt[:, cc * K + k,
                                             oc * 128:(oc + 1) * 128],
                                    rhs=valt[:, hwin * 512:(hwin + 1) * 512],
                                    start=(k == 0 and cc == 0),
                                    stop=(k == K - 1 and cc == 1),
                                )

            for oc in range(2):
                ot = opool.tile([128, HW], f32, tag="ot")
                for hwin in range(2):
                    nc.scalar.copy(ot[:, hwin * 512:(hwin + 1) * 512],
                                   ps[(oc, hwin)][:])
                nc.sync.dma_start(out_d[s][oc * 128:(oc + 1) * 128, :], ot[:])

    nc.compile()
    return nc


def get_nc():
    if "nc" not in _cache:
        _cache["nc"] = _build()
    return _cache["nc"]


def prep_core_inputs(x, offset, weight, core):
    """Host-side shard + layout for one core. x (N,C,H,W) f32,
    offset (N,HW,18) f32, weight (O,C,3,3) f32."""
    s0 = core * SPC
    xbs = np.zeros((SPC, NPIX, C_IN), dtype=_BF16)
    offw = np.empty((SPC, 128, 8, 2 * K), dtype=np.float32)
    for i, s in enumerate(range(s0, s0 + SPC)):
        xt = x[s].reshape(C_IN, HW).T.astype(_BF16)  # (1024, 256)
        xbs[i, :HW] = xt
        offw[i] = offset[s].reshape(8, 128, 2 * K).transpose(1, 0, 2)
    return {"xb": xbs, "off_w": offw}


def make_base_w():
    hwv = (np.arange(8)[None, :] * 128 + np.arange(128)[:, None])  # (128,8)
    ky = np.arange(K) // 3 - 1
    kx = np.arange(K) % 3 - 1
    base = np.empty((128, 8, 2 * K), dtype=np.float32)
    base[:, :, 0::2] = (hwv // W)[:, :, None] + ky[None, None, :]
    base[:, :, 1::2] = (hwv % W)[:, :, None] + kx[None, None, :]
    return base


def make_wt(weight):
    wk = weight.reshape(C_OUT, C_IN, K)  # (O, C, K)
    wt = np.empty((2 * K, 128, C_OUT), dtype=_BF16)
    for cc in range(2):
        for k in range(K):
            wt[cc * K + k] = wk[:, cc * 128:(cc + 1) * 128, k].T
    return wt


def _ensure_device():
    import subprocess
    probe = (
        "import jax, numpy as np; "
        "x = jax.device_put(np.ones((4,4), np.float32), jax.devices()[0]); "
        "print('probe:', float((x+1).sum()))"
    )
    reset = (
        "import ctypes, jax, time; jax.devices(); "
        "lib = ctypes.CDLL('/opt/axon/libaxon_pjrt.so'); "
        "lib.axon_reset.restype = ctypes.c_int64; "
        "print('rc', lib.axon_reset()); time.sleep(2)"
    )
    import sys as _sys
    r = subprocess.run([_sys.executable, "-c", probe], capture_output=True,
                       text=True, timeout=300)
    if "probe: 32.0" in r.stdout:
        return
    for _ in range(3):
        subprocess.run([_sys.executable, "-c", reset], timeout=300)
        r = subprocess.run([_sys.executable, "-c", probe],
                           capture_output=True, text=True, timeout=300)
        if "probe: 32.0" in r.stdout:
            return


def kernel(x, offset, weight):
    from concourse.bass_utils import run_bass_kernel_spmd

    _ensure_device()

    x = np.asarray(x, dtype=np.float32)
    offset = np.asarray(offset, dtype=np.float32)
    weight = np.asarray(weight, dtype=np.float32)
    nc = get_nc()
    base = make_base_w()
    wt = make_wt(weight)
    in_maps = []
    for c in range(NCORES):
        m = prep_core_inputs(x, offset, weight, c)
        m["base_w"] = base
        m["wt"] = wt
        in_maps.append(m)
    res = run_bass_kernel_spmd(nc, in_maps, core_ids=list(range(NCORES)))
    out = np.empty((N, C_OUT, H, W), dtype=np.float32)
    for c in range(NCORES):
        o = np.asarray(res.results[c]["out"], dtype=np.float32)
        out[c * SPC:(c + 1) * SPC] = o.reshape(SPC, C_OUT, H, W)
    return out


# revision 20
# speedup vs baseline: 1.1965x; 1.0721x over previous
"""Deformable 3x3 conv (AdaptiveConv, N=16 C=256 H=W=32) on 8 trn2 cores.

Data-parallel over batch: 2 samples per core. Per sample, on-chip:
  1. DVE pipeline: offsets -> bilinear entry indices + 4 corner weights
     (with OOB remap so clipped entries get the right weights).
  2. idx/weights bounce via DRAM into gather-wrapped / row layouts.
  3. xdup: row-pair duplicated image in DRAM; entry hw = 4 corner pixels
     (2KB bf16) -> one dma_gather descriptor per sampling position.
  4. Transpose-mode dma_gather -> (channel, position) layout tiles.
  5. DVE lerp (broadcast weight rows) -> im2col val tiles.
  6. TensorE: 18 (c-chunk, k) PSUM-accumulated matmuls -> out (o, hw).
"""
from contextlib import ExitStack

import numpy as np

try:
    import ml_dtypes
    _BF16 = ml_dtypes.bfloat16
except ImportError:  # pragma: no cover
    _BF16 = None

N, C_IN, C_OUT, H, W = 16, 256, 256, 32, 32
K = 9
HW = H * W
NPIX = 1152
NCORES = 8
SPC = N // NCORES

_cache = {}


def _build():
    import concourse.bass as bass
    import concourse.mybir as mybir
    import concourse.tile as tile
    from concourse import bacc

    bf = mybir.dt.bfloat16
    f32 = mybir.dt.float32
    i16 = mybir.dt.int16
    AOT = mybir.AluOpType

    nc = bacc.Bacc("TRN2", target_bir_lowering=False, debug=False,
                   num_devices=NCORES)

    xb = nc.declare_dram_parameter("xb", [SPC, NPIX, C_IN], bf, isOutput=False)
    off_w = nc.declare_dram_parameter("off_w", [SPC, 128, 8, 2 * K], f32,
                                      isOutput=False)
    base_w = nc.declare_dram_parameter("base_w", [128, 8, 2 * K], f32,
                                       isOutput=False)
    wt = nc.declare_dram_parameter("wt", [2 * K, 128, C_OUT], bf,
                                   isOutput=False)
    out_d = nc.declare_dram_parameter("out", [SPC, C_OUT, HW], f32,
                                      isOutput=True)

    xdup = nc.dram_tensor("xdup", [SPC, HW, 4 * C_IN], bf)
    istage = nc.dram_tensor("istage", [SPC, HW, K], i16)

    with tile.TileContext(nc) as tc, ExitStack() as ctx:
        cpool = ctx.enter_context(tc.tile_pool(name="const", bufs=1))
        ppool = ctx.enter_context(tc.tile_pool(name="pipe", bufs=2))
        tpool = ctx.enter_context(tc.tile_pool(name="tmp", bufs=2))
        gpool = ctx.enter_context(tc.tile_pool(name="gath", bufs=4))
        vpool = ctx.enter_context(tc.tile_pool(name="val", bufs=2))
        opool = ctx.enter_context(tc.tile_pool(name="outs", bufs=2))
        pspool = ctx.enter_context(
            tc.tile_pool(name="psum", bufs=1, space="PSUM"))
        trpool = ctx.enter_context(
            tc.tile_pool(name="trp", bufs=2, space="PSUM"))

        from concourse.masks import make_identity
        ident = cpool.tile([128, 128], bf)
        make_identity(nc, ident[:])
        baset = cpool.tile([128, 8, 2 * K], f32)
        nc.sync.dma_start(baset[:], base_w[:, :, :])
        wtt = cpool.tile([128, 2 * K, C_OUT], bf)
        nc.sync.dma_start(wtt[:], wt[:, :, :].rearrange("m i o -> i m o"))

        prep = {}
        for s in range(SPC):
            # ---- xdup: entry hw -> [(y,x) (y,x+1) (y+1,x) (y+1,x+1)] ----
            xs = xb[s]
            for half in range(2):
                src = bass.AP(xs.tensor, xs.offset + half * W * C_IN,
                              [[C_IN, HW], [1, 2 * C_IN]])
                nc.scalar.dma_start(
                    xdup[s][:, half * 2 * C_IN:(half + 1) * 2 * C_IN], src)

            # ---- pipeline: (128, 8, 18) wrapped layout ----
            SH = [128, 8, 2 * K]
            offt = ppool.tile(SH, f32, tag="off")
            nc.sync.dma_start(offt[:], off_w[s])
            py = tpool.tile(SH, f32, tag="py")
            nc.vector.tensor_tensor(py[:], offt[:], baset[:], AOT.add)
            fli = tpool.tile(SH, mybir.dt.int16, tag="fli")
            nc.vector.tensor_copy(fli[:], py[:])
            cf = tpool.tile(SH, f32, tag="cf")
            nc.vector.tensor_copy(cf[:], fli[:])
            gg = tpool.tile(SH, f32, tag="gg")
            nc.vector.tensor_tensor(gg[:], cf[:], py[:], AOT.is_gt)
            fl = tpool.tile(SH, f32, tag="fl")
            nc.vector.tensor_tensor(fl[:], cf[:], gg[:], AOT.subtract)
            frac = tpool.tile(SH, f32, tag="frac")
            nc.vector.tensor_tensor(frac[:], py[:], fl[:], AOT.subtract)
            a = tpool.tile(SH, f32, tag="a")           # [fl >= 0]
            nc.vector.tensor_scalar(a[:], fl[:], 0.0, None, AOT.is_ge)
            vb = tpool.tile(SH, f32, tag="vb")
            nc.vector.tensor_scalar(vb[:], fl[:], 31.0, None, AOT.is_le)
            v0 = tpool.tile(SH, f32, tag="v0")         # fl in [0,31]
            nc.vector.tensor_tensor(v0[:], a[:], vb[:], AOT.mult)
            va = tpool.tile(SH, f32, tag="va")
            nc.vector.tensor_scalar(va[:], fl[:], -1.0, None, AOT.is_ge)
            nc.vector.tensor_scalar(vb[:], fl[:], 30.0, None, AOT.is_le)
            v1 = tpool.tile(SH, f32, tag="v1")         # fl+1 in [0,31]
            nc.vector.tensor_tensor(v1[:], va[:], vb[:], AOT.mult)
            flc = tpool.tile(SH, f32, tag="flc")
            nc.vector.tensor_scalar(flc[:], fl[:], 0.0, 31.0, AOT.max,
                                    AOT.min)
            om = tpool.tile(SH, f32, tag="om")         # 1 - frac
            nc.vector.tensor_scalar(om[:], frac[:], -1.0, 1.0, AOT.mult,
                                    AOT.add)
            w0 = tpool.tile(SH, f32, tag="w0")         # lo-corner weight
            nc.vector.tensor_tensor(w0[:], om[:], v0[:], AOT.mult)
            w1 = tpool.tile(SH, f32, tag="w1")         # hi-corner weight
            nc.vector.tensor_tensor(w1[:], frac[:], v1[:], AOT.mult)
            # OOB remap onto clipped entry: wP0 = a*w0 + (1-a)*w1, wP1 = a*w1
            na = tpool.tile(SH, f32, tag="na")
            nc.vector.tensor_scalar(na[:], a[:], -1.0, 1.0, AOT.mult, AOT.add)
            t0 = tpool.tile(SH, f32, tag="t0")
            nc.vector.tensor_tensor(t0[:], a[:], w0[:], AOT.mult)
            t1 = tpool.tile(SH, f32, tag="t1")
            nc.vector.tensor_tensor(t1[:], na[:], w1[:], AOT.mult)
            wp0 = tpool.tile(SH, f32, tag="wp0")
            nc.vector.tensor_tensor(wp0[:], t0[:], t1[:], AOT.add)
            wp1 = tpool.tile(SH, f32, tag="wp1")
            nc.vector.tensor_tensor(wp1[:], a[:], w1[:], AOT.mult)

            wplane = ppool.tile([128, 8, K, 4], f32, tag="wplane")
            wy0, wx0 = wp0[:, :, 0::2], wp0[:, :, 1::2]
            wy1, wx1 = wp1[:, :, 0::2], wp1[:, :, 1::2]
            nc.vector.tensor_tensor(wplane[:, :, :, 0], wy0, wx0, AOT.mult)
            nc.vector.tensor_tensor(wplane[:, :, :, 1], wy0, wx1, AOT.mult)
            nc.vector.tensor_tensor(wplane[:, :, :, 2], wy1, wx0, AOT.mult)
            nc.vector.tensor_tensor(wplane[:, :, :, 3], wy1, wx1, AOT.mult)

            t2 = tpool.tile([128, 8, K], f32, tag="t2")
            nc.vector.tensor_scalar(t2[:], flc[:, :, 0::2], float(W), None,
                                    AOT.mult)
            idxp = ppool.tile([128, 8, K], i16, tag="idxp")
            nc.vector.tensor_tensor(idxp[:], t2[:], flc[:, :, 1::2], AOT.add)

            # ---- bounce idx/weights through DRAM (hw-major staging) ----
            ist = istage[s]  # (HW, K): addr = hw*K + k
            nc.scalar.dma_start(
                bass.AP(ist.tensor, ist.offset,
                        [[K, 128], [128 * K, 8], [1, K]]),
                idxp[:])
            # wrapped idx: load (16, 64, K) then ACT repack+replicate per group
            idxwt = ppool.tile([16, HW // 16, K], i16, tag="idxwt")
            nc.scalar.dma_start(
                idxwt[:],
                bass.AP(ist.tensor, ist.offset,
                        [[K, 16], [16 * K, HW // 16], [1, K]]))
            idxw16 = ppool.tile([16, K, HW // 16], i16, tag="idxw16")
            nc.scalar.copy(
                idxw16[:],
                bass.AP(idxwt.tensor, idxwt.offset,
                        [[idxwt.ap[0][0], 16], [1, K], [K, HW // 16]]))
            idxw = ppool.tile([128, K, HW // 16], i16, tag="idxw")
            for grp in range(8):
                nc.scalar.dma_start(idxw[grp * 16:(grp + 1) * 16], idxw16[:])

            prep[s] = (idxw, wplane)

        for s in range(SPC):
            idxw, wplane = prep[s]
            ps = {}
            for oc in range(2):
                for hwin in range(2):
                    pst = pspool.tile([128, 512], f32, tag=f"ps{oc}{hwin}")
                    ps[(oc, hwin)] = pst

            # ---- per-k gather; lerp + PE transpose per tap ----
            for k0 in range(0, K):
                g2 = gpool.tile([128, 8, 4 * C_IN], bf, tag="g2")
                nc.gpsimd.dma_gather(
                    out_ap=g2[:],
                    in_ap=xdup[s],
                    idxs_ap=idxw[:, k0, :],
                    num_idxs=HW,
                    num_idxs_reg=HW,
                    elem_size=4 * C_IN,
                    transpose=False,
                    single_packet=False,
                )
                for k in (k0,):
                    g = g2[:]
                    ptr = {}
                    for cc in range(2):
                        pt = trpool.tile([128, 8, 128], bf, tag=f"tr{cc}")
                        ptr[cc] = pt
                    # batched lerp: one bcast-mult + 3 adds for the whole k
                    wsl = wplane[:, :, k, :]
                    wb = bass.AP(wsl.tensor, wsl.offset,
                                 list(wsl.ap) + [[0, C_IN]])
                    mall = vpool.tile([128, 8, 4, C_IN], bf, tag="mall")
                    nc.vector.tensor_tensor(
                        mall[:],
                        g.rearrange("p b (c4 c) -> p b c4 c", c=C_IN),
                        wb, AOT.mult)
                    a1 = vpool.tile([128, 8, C_IN], bf, tag="a1")
                    nc.vector.tensor_tensor(a1[:], mall[:, :, 0, :],
                                            mall[:, :, 1, :], AOT.add)
                    a2 = vpool.tile([128, 8, C_IN], bf, tag="a2")
                    nc.vector.tensor_tensor(a2[:], mall[:, :, 2, :],
                                            mall[:, :, 3, :], AOT.add)
                    vall = vpool.tile([128, 8, C_IN], bf, tag="vall")
                    nc.vector.tensor_tensor(vall[:], a1[:], a2[:], AOT.add)
                    for b in range(8):
                        for cc in range(2):
                            nc.tensor.transpose(
                                ptr[cc][:, b, :],
                                vall[:, b, cc * 128:(cc + 1) * 128], ident[:])
                    for cc in range(2):
                        valt = vpool.tile([128, HW], bf, tag="valt")
                        nc.scalar.copy(
                            valt[:], ptr[cc][:].rearrange("p a b -> p (a b)"))
                        for oc in range(2):
                            for hwin in range(2):
                                nc.tensor.matmul(
                                    ps[(oc, hwin)][:],
                                    lhsT=wtt[:, cc * K + k,
                                             oc * 128:(oc + 1) * 128],
                                    rhs=valt[:, hwin * 512:(hwin + 1) * 512],
                                    start=(k == 0 and cc == 0),
                                    stop=(k == K - 1 and cc == 1),
                                )

            for oc in range(2):
                ot = opool.tile([128, HW], f32, tag="ot")
                for hwin in range(2):
                    nc.scalar.copy(ot[:, hwin * 512:(hwin + 1) * 512],
                                   ps[(oc, hwin)][:])
                nc.sync.dma_start(out_d[s][oc * 128:(oc + 1) * 128, :], ot[:])

    nc.compile()
    return nc


def get_nc():
    if "nc" not in _cache:
        _cache["nc"] = _build()
    return _cache["nc"]


def prep_core_inputs(x, offset, weight, core):
    """Host-side shard + layout for one core. x (N,C,H,W) f32,
    offset (N,HW,18) f32, weight (O,C,3,3) f32."""
    s0 = core * SPC
    xbs = np.zeros((SPC, NPIX, C_IN), dtype=_BF16)
    offw = np.empty((SPC, 128, 8, 2 * K), dtype=np.float32)
    for i, s in enumerate(range(s0, s0 + SPC)):
        xt = x[s].reshape(C_IN, HW).T.astype(_BF16)  # (1024, 256)
        xbs[i, :HW] = xt
        offw[i] = offset[s].reshape(8, 128, 2 * K).transpose(1, 0, 2)
    return {"xb": xbs, "off_w": offw}


def make_base_w():
    hwv = (np.arange(8)[None, :] * 128 + np.arange(128)[:, None])  # (128,8)
    ky = np.arange(K) // 3 - 1
    kx = np.arange(K) % 3 - 1
    base = np.empty((128, 8, 2 * K), dtype=np.float32)
    base[:, :, 0::2] = (hwv // W)[:, :, None] + ky[None, None, :]
    base[:, :, 1::2] = (hwv % W)[:, :, None] + kx[None, None, :]
    return base


def make_wt(weight):
    wk = weight.reshape(C_OUT, C_IN, K)  # (O, C, K)
    wt = np.empty((2 * K, 128, C_OUT), dtype=_BF16)
    for cc in range(2):
        for k in range(K):
            wt[cc * K + k] = wk[:, cc * 128:(cc + 1) * 128, k].T
    return wt


def _ensure_device():
    import subprocess
    probe = (
        "import jax, numpy as np; "
        "x = jax.device_put(np.ones((4,4), np.float32), jax.devices()[0]); "
        "print('probe:', float((x+1).sum()))"
    )
    reset = (
        "import ctypes, jax, time; jax.devices(); "
        "lib = ctypes.CDLL('/opt/axon/libaxon_pjrt.so'); "
        "lib.axon_reset.restype = ctypes.c_int64; "
        "print('rc', lib.axon_reset()); time.sleep(2)"
    )
    import sys as _sys
    r = subprocess.run([_sys.executable, "-c", probe], capture_output=True,
                       text=True, timeout=300)
    if "probe: 32.0" in r.stdout:
        return
    for _ in range(3):
        subprocess.run([_sys.executable, "-c", reset], timeout=300)
        r = subprocess.run([_sys.executable, "-c", probe],
                           capture_output=True, text=True, timeout=300)
        if "probe: 32.0" in r.stdout:
            return


def kernel(x, offset, weight):
    from concourse.bass_utils import run_bass_kernel_spmd

    _ensure_device()

    x = np.asarray(x, dtype=np.float32)
    offset = np.asarray(offset, dtype=np.float32)
    weight = np.asarray(weight, dtype=np.float32)
    nc = get_nc()
    base = make_base_w()
    wt = make_wt(weight)
    in_maps = []
    for c in range(NCORES):
        m = prep_core_inputs(x, offset, weight, c)
        m["base_w"] = base
        m["wt"] = wt
        in_maps.append(m)
    res = run_bass_kernel_spmd(nc, in_maps, core_ids=list(range(NCORES)))
    out = np.empty((N, C_OUT, H, W), dtype=np.float32)
    for c in range(NCORES):
        o = np.asarray(res.results[c]["out"], dtype=np.float32)
        out[c * SPC:(c + 1) * SPC] = o.reshape(SPC, C_OUT, H, W)
    return out
